# revision 1
# baseline (speedup 1.0000x reference)
"""Bahdanau-style attention kernel for Trainium2, data-parallel over batch
across 8 NeuronCores.  v3: masked rows are skipped entirely -- the host
computes, per batch, the indices of rows with mask==1 and the device
gathers only those rows (padded to a static budget S_P).  Since
exp(-1e10 + x) underflows to exactly 0, masked rows contribute nothing to
softmax or context, so the packed computation is exact.

Reference computation (per batch b):
    W_h, W_e = W_attn[:H], W_attn[H:]
    proj   = hidden @ W_h + enc[b] @ W_e + b_attn          # [S, H]
    energy = tanh(proj)
    scores = energy @ W_v                                   # [S]
    scores = where(mask==0, -1e10, scores)
    attn   = softmax(scores)
    ctx    = attn @ enc[b]                                  # [2H]

Shapes: B=32, S=1024, H=512, 2H=1024.  8 cores x 4 batches each.
With a Bernoulli(0.5) mask, S_P is typically 640 (5 s-tiles vs 8).

Numerics: f32r matmul datapath; scoring chain in bf16 on the DVE; softmax
and outputs f32.  Expected rel err ~2e-3 (tolerance 2e-2).
"""

import numpy as np

B, S, H = 32, 1024, 512
E = 2 * H            # 1024
N_CORES = 8
B_LOC = B // N_CORES  # 4
ET = E // 128         # 8 e-tiles (k-tiles of the main matmul)
KT_H = H // 128       # 4 k-tiles for hidden @ W_h
NEG = -1e10

_cache = {}


def _install_tile_drain_patch():
    """walrus in this container rejects >1 sem-wait on the SP CTRL drain that
    TileContext emits at kernel tail; split the waits across 1-wait nops."""
    import concourse.tile as tile
    import concourse.mybir as mybir
    from concourse.vector_clock import ScopedClock

    if getattr(tile.TileContext, "_drain_patch_installed", False):
        return

    def _drain_and_barrier_split(self, tick_clock, wait_clock):
        nc = self.nc
        probe = nc.sync.nop(nofuse=True, hint="tail_wait_probe")
        wait_clock.add_sem_waits(
            probe.ins, ScopedClock({None: tick_clock.global_clock})
        )
        si = probe.ins.sync_info
        waits = list(si.on_wait) if si and si.on_wait else []
        if len(waits) > 1:
            si.on_wait = waits[:1]
            for w in waits[1:]:
                n = nc.sync.nop(nofuse=True, hint="tail_wait_extra")
                nsi = n.ins.sync_info
                if nsi is None:
                    n.ins.sync_info = mybir.SyncInfo(on_wait=[w], on_update=[])
                else:
                    nsi.on_wait = [w]
        nc.sync.drain()
        nc.all_engine_barrier()
        assert self.sems is not None
        popped = nc._tile_sem_poison_stack.pop()
        assert popped is self._sem_poison
        # chunked clear_and_free_semaphores: walrus rejects RANGE_CLEAR ISA
        # instructions spanning more than a few semaphores ("ISA wrong
        # length"), so clear in <=3-wide ranges.
        sems = list(self.sems.allocated().values())
        sem_nums = sorted(s.num if hasattr(s, "num") else s for s in sems)
        if sem_nums:
            runs = []
            lo = prev = sem_nums[0]
            for n in sem_nums[1:]:
                if n == prev + 1:
                    prev = n
                else:
                    runs.append((lo, prev))
                    lo = prev = n
            runs.append((lo, prev))
            for lo, hi in runs:
                for c0 in range(lo, hi + 1, 3):
                    c1 = min(c0 + 2, hi)
                    r = range(c0, c1 + 1)
                    assert nc._state.free_isdisjoint(r)
                    nc.gpsimd.dma_reset(r)
                    nc.gpsimd.sem_clear(r)
            nc._state.prepend_free_semaphores(sem_nums)
            for poison_set in nc._tile_sem_poison_stack:
                poison_set.update(sem_nums)
        nc.all_engine_barrier()

    tile.TileContext._drain_and_barrier = _drain_and_barrier_split
    tile.TileContext._drain_patch_installed = True


def _split_multiwaits(nc, max_waits=1):
    """walrus's setupSyncWait rejects instructions carrying more than a couple
    of semaphore waits.  Move excess waits onto same-engine nops inserted
    immediately before the offending instruction (engine executes in order, so
    semantics are identical)."""
    import concourse.mybir as mybir

    for f in nc.m.functions:
        for bb in f.blocks:
            out = []
            for inst in bb.instructions:
                si = inst.sync_info
                waits = list(si.on_wait) if si and si.on_wait else []
                lim = max_waits
                if len(waits) > lim:
                    excess = waits[:-lim]
                    si.on_wait = waits[-lim:]
                    for i in range(0, len(excess), max_waits):
                        nop = mybir.InstNoOp(
                            name=f"I-{nc.next_id()}-waitsplit", ins=[], outs=[]
                        )
                        nop.engine = inst.engine
                        nop.sync_info = mybir.SyncInfo(
                            on_wait=excess[i:i + max_waits], on_update=[]
                        )
                        nc.register_instruction(nop, overwrite=True)
                        out.append(nop)
                out.append(inst)
            bb.instructions[:] = out


def _emit_tail(nc, pools, pends, ctx_d, attn_d, ST_P):
    """Deferred softmax-tail + context emission for pending batches, placed
    inside the NEXT batch's score phase so the in-order PE queue never stalls
    on the current batch's softmax chain.

    ctx(b) = (p_r @ enc_b) / denom, accumulated per E-half in a PSUM bank
    (f32r matmuls must write partition 0)."""
    import concourse.mybir as mybir
    f32 = mybir.dt.float32
    pctx_pool, psmall_pool, ctxpool, bpool, ones_col_f, ones_row_f = pools
    for (b, enc_b, p_r, p_exp, rowsum) in pends:
        ctx_sb = ctxpool.tile([1, E], f32, tag="ctx_sb")
        # denominator chain
        p_den = psmall_pool.tile([1, 1], f32, tag="small")
        nc.tensor.matmul(p_den[:], rowsum[:], ones_col_f[:],
                         start=True, stop=True)
        rd = bpool.tile([1, 1], f32, tag="rd")
        nc.vector.reciprocal(rd[:], p_den[:])
        p_rb = psmall_pool.tile([128, 1], f32, tag="small")
        nc.tensor.matmul(p_rb[:], ones_row_f[:], rd[:],
                         start=True, stop=True)
        rb = bpool.tile([128, 1], f32, tag="rb")
        nc.any.tensor_copy(rb[:], p_rb[:])
        # ctx matmuls: p_r (stationary, M=1) x enc (moving, N=512)
        for h2 in range(2):
            p_c = pctx_pool.tile([1, 512], f32, tag="p_c")
            for st in range(ST_P):
                nc.tensor.matmul(
                    p_c[:], p_r[:, st:st + 1],
                    enc_b[:, st * E + 512 * h2: st * E + 512 * (h2 + 1)],
                    start=(st == 0), stop=(st == ST_P - 1),
                )
            nc.vector.tensor_scalar_mul(
                ctx_sb[:, 512 * h2:512 * (h2 + 1)], p_c[:], rd[:])
        nc.sync.dma_start(ctx_d[b][None, :], ctx_sb[:])
        # attention output (packed)
        attn_sb = bpool.tile([128, ST_P], f32, tag="attn_sb")
        nc.vector.tensor_scalar_mul(attn_sb[:], p_exp[:], rb[:])
        nc.sync.dma_start(
            attn_d[b].rearrange("(st p) -> p st", p=128), attn_sb[:]
        )


def build_kernel(n_iters: int = 1, s_p: int = 640):
    """Build the per-core Bass program for packed row count s_p (a multiple
    of 128).  n_iters>1 repeats the whole compute body (for slope-based
    timing); outputs are just rewritten."""
    _install_tile_drain_patch()
    import concourse.bass as bass
    import concourse.tile as tile
    import concourse.mybir as mybir
    from concourse.mybir import AluOpType as alu
    from concourse.mybir import ActivationFunctionType as act
    from concourse.library_config import mlp as mlp_lib

    f32 = mybir.dt.float32
    f32r = mybir.dt.float32r
    bf16 = mybir.dt.bfloat16
    i16 = mybir.dt.int16

    ST_P = s_p // 128
    IDXW = s_p // 16  # idx columns per batch

    nc = bass.Bass("TRN2", target_bir_lowering=False, debug=False,
                   num_devices=N_CORES)

    hidden_d = nc.dram_tensor("hidden", [B_LOC, H], f32r, kind="ExternalInput").ap()
    enc_d = nc.dram_tensor("enc", [B_LOC, S, E], f32r, kind="ExternalInput").ap()
    gidx_d = nc.dram_tensor("gidx", [128, B_LOC * IDXW], i16, kind="ExternalInput").ap()
    pbias_d = nc.dram_tensor("pbias", [B_LOC, s_p], f32, kind="ExternalInput").ap()
    wattn_d = nc.dram_tensor("w_attn", [3 * H, H], f32r, kind="ExternalInput").ap()
    battn_d = nc.dram_tensor("b_attn", [H], f32r, kind="ExternalInput").ap()
    wv_d = nc.dram_tensor("w_v", [H], f32, kind="ExternalInput").ap()
    ctx_d = nc.dram_tensor("out_ctx", [B_LOC, E], f32, kind="ExternalOutput").ap()
    attn_d = nc.dram_tensor("out_attn", [B_LOC, s_p], f32, kind="ExternalOutput").ap()

    with tile.TileContext(nc) as tc:
        with (
            tc.tile_pool(name="const", bufs=1) as cpool,
            tc.tile_pool(name="enc", bufs=3) as encpool,
            tc.tile_pool(name="encT", bufs=7) as encTpool,
            tc.tile_pool(name="work", bufs=4) as wpool,
            tc.tile_pool(name="perb", bufs=3) as bpool,
            tc.tile_pool(name="ctxp", bufs=2) as ctxpool,
            tc.tile_pool(name="ptr", bufs=3, space="PSUM") as ptr_pool,
            tc.tile_pool(name="pproj", bufs=3, space="PSUM") as pproj_pool,
            tc.tile_pool(name="pctx", bufs=1, space="PSUM") as pctx_pool,
            tc.tile_pool(name="psmall", bufs=1, space="PSUM") as psmall_pool,
        ):
            # ---------------- constants (Pool work first) ----------------
            ones_f = cpool.tile([128, 128], f32)
            nc.vector.memset(ones_f[:], 1.0)
            id_f = cpool.tile([128, 128], f32)
            nc.gpsimd.affine_select(
                id_f[:], ones_f[:], pattern=[[1, 128]],
                compare_op=alu.is_equal, fill=0.0, base=0,
                channel_multiplier=-1,
            )
            id_sb = cpool.tile([128, 128], f32r)
            nc.vector.tensor_copy(id_sb[:], id_f[:])
            # sel built on the DVE (column b of the identity as a
            # per-partition scalar times ones) so the Pool queue reaches
            # load_library + the batch-0 gathers ~1.1us earlier.
            sel_f = cpool.tile([B_LOC, B_LOC * 128], f32)
            for b in range(B_LOC):
                nc.vector.tensor_scalar_mul(
                    sel_f[:, b * 128:(b + 1) * 128], ones_f[0:B_LOC, :],
                    id_f[0:B_LOC, b:b + 1],
                )
            sel = cpool.tile([B_LOC, B_LOC * 128], f32r)
            nc.vector.tensor_copy(sel[:], sel_f[:])

            # gather ucode library (InstDMAGatherAnt lives in 'mlp');
            # Pool-engine affine_selects must be emitted BEFORE this.
            nc.gpsimd.load_library(mlp_lib)

            # gather indices for all batches (tiny, HWDGE)
            gidx_sb = cpool.tile([128, B_LOC * IDXW], i16)
            nc.sync.dma_start(gidx_sb[:], gidx_d)

            # one shared register for the gather count (a fresh to_reg per
            # gather exhausts the register pool for large n_iters)
            nidx_reg = nc.gpsimd.to_reg(128)

            def load_enc(dst, b):
                for c in range(ST_P):
                    nc.gpsimd.dma_gather(
                        dst[:, c * E:(c + 1) * E].rearrange(
                            "p (o e) -> p o e", o=1),
                        enc_d[b],
                        gidx_sb[:, b * IDXW + c * 8: b * IDXW + (c + 1) * 8],
                        num_idxs=128,
                        num_idxs_reg=nidx_reg,
                        elem_size=E,
                    )

            # Weights ride the ACT HWDGE ring (separate from both the SP
            # ring carrying gidx/hidden and the SWDGE gather queue), so the
            # batch-0 gather chunks aren't queued behind 3MB of weights.
            w_e = cpool.tile([128, ET * H], f32r)
            w_h = cpool.tile([128, KT_H * H], f32r)
            nc.scalar.dma_start(
                w_h[:].rearrange("p (kt h) -> p kt h", h=H),
                wattn_d[0:H, :].rearrange("(kt p) h -> p kt h", p=128),
            )
            nc.scalar.dma_start(
                w_e[:].rearrange("p (kt h) -> p kt h", h=H),
                wattn_d[H:3 * H, :].rearrange("(kt p) h -> p kt h", p=128),
            )

            enc_first = encpool.tile([128, ST_P * E], f32r, tag="enc_b")
            load_enc(enc_first, 0)

            # ---------------- weights / small inputs ----------------
            ones_col_f = cpool.tile([128, 1], f32)
            nc.vector.memset(ones_col_f[:], 1.0)
            ones_row_f = cpool.tile([1, 128], f32)
            nc.vector.memset(ones_row_f[:], 1.0)
            ones_row_r = cpool.tile([1, 128], f32r)
            nc.vector.tensor_copy(ones_row_r[:], ones_row_f[:])

            hidT = cpool.tile([128, KT_H * B_LOC], f32r)
            for kt in range(KT_H):
                nc.sync.dma_start(
                    hidT[:, kt * B_LOC:(kt + 1) * B_LOC],
                    hidden_d[:, kt * 128:(kt + 1) * 128].rearrange("b p -> p b"),
                )
            wv_row = cpool.tile([1, H], f32)
            nc.sync.dma_start(wv_row[:], wv_d[None, :])
            battn_row = cpool.tile([1, H], f32r)
            nc.sync.dma_start(battn_row[:], battn_d[None, :])
            # padding bias: 0 on live rows, -1e10 on padded rows
            pb_sb = cpool.tile([128, B_LOC * ST_P], f32)
            for b in range(B_LOC):
                nc.sync.dma_start(
                    pb_sb[:, b * ST_P:(b + 1) * ST_P],
                    pbias_d[b].rearrange("(st p) -> p st", p=128),
                )

            wv_bc = cpool.tile([128, H], bf16)
            hb_bc = cpool.tile([128, B_LOC * H], f32)

            def emit_preamble_mms():
                """PE matmuls of the preamble (W_v broadcast, proj_h, hb
                broadcast), deferred past the first transposes so the
                in-order PE queue isn't blocked on the w_h/hidden loads."""
                p_wv = psmall_pool.tile([128, H], f32, tag="small")
                nc.tensor.matmul(p_wv[:], ones_row_f[:], wv_row[:],
                                 start=True, stop=True)
                nc.any.tensor_copy(wv_bc[:], p_wv[:])
                p_ph = psmall_pool.tile([B_LOC, H], f32, tag="small")
                for kt in range(KT_H):
                    nc.tensor.matmul(
                        p_ph[:],
                        hidT[:, kt * B_LOC:(kt + 1) * B_LOC],
                        w_h[:, kt * H:(kt + 1) * H],
                        start=(kt == 0), stop=False,
                    )
                nc.tensor.matmul(p_ph[:], ones_row_r[:, 0:B_LOC],
                                 battn_row[:], start=False, stop=True)
                hb = cpool.tile([B_LOC, H], f32r)
                nc.any.tensor_copy(hb[:], p_ph[:])
                for b in range(B_LOC):
                    p_hb = psmall_pool.tile([128, H], f32, tag="small")
                    nc.tensor.matmul(p_hb[:], sel[:, b * 128:(b + 1) * 128],
                                     hb[:], start=True, stop=True)
                    nc.any.tensor_copy(hb_bc[:, b * H:(b + 1) * H], p_hb[:])

            # ---------------- main loop ----------------
            preamble_emitted = False
            emit_pools = (pctx_pool, psmall_pool, ctxpool, bpool,
                          ones_col_f, ones_row_f)
            for it in range(n_iters):
                pend = []
                for b in range(B_LOC):
                    if it == 0 and b == 0:
                        enc_b = enc_first
                    else:
                        enc_b = enc_next
                    nb = (b + 1) % B_LOC
                    if not (it == n_iters - 1 and b == B_LOC - 1):
                        enc_next = encpool.tile([128, ST_P * E], f32r,
                                                tag="enc_b")
                        load_enc(enc_next, nb)

                    s_sb = bpool.tile([128, ST_P], f32, tag="s_sb")
                    encT_q = []
                    for st in range(ST_P + 1):
                        if st < ST_P:
                            encT = encTpool.tile([128, E], f32r, tag="encT")
                            for g in range(2):
                                p_tr = ptr_pool.tile([128, 512], f32r, tag="p_tr")
                                for j4 in range(4):
                                    j = g * 4 + j4
                                    nc.tensor.transpose(
                                        p_tr[:, j4 * 128:(j4 + 1) * 128],
                                        enc_b[:, st * E + j * 128: st * E + (j + 1) * 128],
                                        id_sb[:],
                                    )
                                nc.any.tensor_copy(
                                    encT[:, g * 512:(g + 1) * 512], p_tr[:]
                                )
                            encT_q.append(encT)
                        if st == 0:
                            continue
                        stp = st - 1
                        if not preamble_emitted:
                            emit_preamble_mms()
                            preamble_emitted = True
                        encT_p = encT_q.pop(0)
                        p_proj = pproj_pool.tile([128, H], f32, tag="p_proj")
                        for j in range(ET):
                            nc.tensor.matmul(
                                p_proj[:],
                                encT_p[:, j * 128:(j + 1) * 128],
                                w_e[:, j * H:(j + 1) * H],
                                start=(j == 0), stop=(j == ET - 1),
                            )
                        # energy = tanh(p_proj + hb_bc[b] + pbias); the pad
                        # bias is folded in at the score level below instead.
                        ein = wpool.tile([128, H], f32, tag="ein")
                        nc.vector.tensor_add(
                            ein[:], p_proj[:], hb_bc[:, b * H:(b + 1) * H]
                        )
                        energy = wpool.tile([128, H], bf16, tag="energy")
                        nc.scalar.activation(energy[:], ein[:], act.Tanh)
                        scr = wpool.tile([128, H], bf16, tag="scr")
                        nc.vector.tensor_mul(scr[:], energy[:], wv_bc[:])
                        nc.vector.reduce_sum(
                            s_sb[:, stp:stp + 1], scr[:], axis=mybir.AxisListType.X
                        )
                        if stp == min(3, ST_P - 1) and pend:
                            _emit_tail(nc, emit_pools, pend, ctx_d, attn_d,
                                       ST_P)
                            pend = []

                    # ---- softmax (DVE/ACT part; PE part deferred) ----
                    sm = bpool.tile([128, ST_P], f32, tag="sm")
                    nc.vector.tensor_add(
                        sm[:], s_sb[:], pb_sb[:, b * ST_P:(b + 1) * ST_P]
                    )
                    p_exp = bpool.tile([128, ST_P], f32, tag="p_exp")
                    rowsum = bpool.tile([128, 1], f32, tag="rowsum")
                    nc.scalar.activation(p_exp[:], sm[:], act.Exp,
                                         accum_out=rowsum[:])
                    p_r = bpool.tile([128, ST_P], f32r, tag="p_r")
                    nc.any.tensor_copy(p_r[:], p_exp[:])
                    pend.append((b, enc_b, p_r, p_exp, rowsum))
                _emit_tail(nc, emit_pools, pend, ctx_d, attn_d, ST_P)
                pend = []

    _split_multiwaits(nc)
    import concourse.mybir as mybir2
    mybir2.codegen_inst_isa_subclasses(nc)
    return nc


def _get_nc(n_iters: int = 1, s_p: int = 640):
    key = ("nc", n_iters, s_p)
    if key not in _cache:
        _cache[key] = build_kernel(n_iters, s_p)
    return _cache[key]


def _pack_core(mask_c, s_p):
    """Per-core gather indices + padding bias + scatter info."""
    idxw = s_p // 16
    gidx = np.zeros((128, B_LOC * idxw), np.int16)
    pbias = np.zeros((B_LOC, s_p), np.float32)
    scat = []
    for b in range(B_LOC):
        idx = np.nonzero(mask_c[b])[0].astype(np.int16)
        n = len(idx)
        pad = np.zeros(s_p, np.int16)
        pad[:n] = idx
        pbias[b, n:] = NEG
        for c in range(8):
            w = pad.reshape(idxw, 16).T  # [16, idxw]: i -> [i%16, i//16]
            gidx[16 * c:16 * (c + 1), b * idxw:(b + 1) * idxw] = w
        scat.append((n, pad[:n].astype(np.int64)))
    return gidx, pbias, scat


def shard_inputs(hidden, encoder_outputs, mask, W_attn, b_attn, W_v,
                 s_p=None):
    hidden = np.ascontiguousarray(np.asarray(hidden, dtype=np.float32))
    enc = np.ascontiguousarray(np.asarray(encoder_outputs, dtype=np.float32))
    mask = np.ascontiguousarray(np.asarray(mask, dtype=np.int32))
    W_attn = np.ascontiguousarray(np.asarray(W_attn, dtype=np.float32))
    b_attn = np.ascontiguousarray(np.asarray(b_attn, dtype=np.float32))
    W_v = np.ascontiguousarray(np.asarray(W_v, dtype=np.float32))
    if s_p is None:
        s_p = pick_s_p(mask)
    in_maps, scats = [], []
    for c in range(N_CORES):
        sl = slice(c * B_LOC, (c + 1) * B_LOC)
        gidx, pbias, scat = _pack_core(mask[sl], s_p)
        in_maps.append({
            "hidden": hidden[sl],
            "enc": enc[sl],
            "gidx": gidx,
            "pbias": pbias,
            "w_attn": W_attn,
            "b_attn": b_attn,
            "w_v": W_v,
        })
        scats.append(scat)
    return in_maps, scats, s_p


def pick_s_p(mask):
    counts = mask.reshape(B, S).sum(1)
    m = int(counts.max())
    return max(128, min(S, ((m + 127) // 128) * 128))


def kernel(hidden, encoder_outputs, mask, W_attn, b_attn, W_v):
    from concourse.bass_utils import run_bass_kernel_spmd

    in_maps, scats, s_p = shard_inputs(
        hidden, encoder_outputs, mask, W_attn, b_attn, W_v)
    nc = _get_nc(1, s_p)
    res = run_bass_kernel_spmd(nc, in_maps, list(range(N_CORES)))
    context = np.concatenate([res.results[c]["out_ctx"] for c in range(N_CORES)], 0)
    attn_p = np.concatenate([res.results[c]["out_attn"] for c in range(N_CORES)], 0)
    attn_w = np.zeros((B, S), np.float32)
    for c in range(N_CORES):
        for b in range(B_LOC):
            n, idx = scats[c][b]
            attn_w[c * B_LOC + b, idx] = attn_p[c * B_LOC + b, :n]
    return context.astype(np.float32), attn_w.astype(np.float32)

